# revision 6
# baseline (speedup 1.0000x reference)
"""Trainium2 Bass kernel for nn_Decoder_86242943304025.

Pointer-network decoder: LSTM cell + glimpse attention + pointer attention +
masked categorical sampling, 256 sequential steps, B=1024 sharded over 8
NeuronCores (128 batch rows per core = SBUF partition dim).

Host side precomputes (exact jax/CPU, matching the reference bit-for-bit):
  - e_g / e_p projected refs (step-invariant)
  - Gumbel noise for every step (jax.random.categorical internals, key 42)
and packs per-core streaming layouts. The device kernel runs the full
recurrence; sampling = argmax(logp + gumbel) with first-occurrence
tie-breaking to replicate jnp.argmax.
"""
import os
import sys
import numpy as np
from contextlib import ExitStack

for _p in ("/opt/trn_rl_repo", "/root/.axon_site", "/root/.axon_site/_ro/trn_rl_repo",
           "/root/.axon_site/_ro/pypackages"):
    if os.path.isdir(_p) and _p not in sys.path:
        sys.path.append(_p)

import concourse.bass as bass
import concourse.bacc as bacc
import concourse.tile as tile
import concourse.mybir as mybir
from concourse.bass_utils import run_bass_kernel_spmd

F32 = mybir.dt.float32
I32 = mybir.dt.int32
U32 = mybir.dt.uint32
AF = mybir.ActivationFunctionType
OP = mybir.AluOpType

NCORES = 8
B, L, E, H = 1024, 256, 256, 256
BL = B // NCORES            # 128 batch rows per core
H4 = 4 * H
C_TANH = 10.0
NEG = -1.0e30
L_BLK = 16                  # l (or h) rows per streamed chunk
NCHUNK = L // L_BLK         # 16 chunks of [128, L_BLK*256] fp32 (2 MB)
CHW = L_BLK * H             # chunk free width = 4096


def build_program(n_steps=L, use_act_sigmoid=False, stream_bufs=3):
    nc = bacc.Bacc("TRN2", target_bir_lowering=False, debug=False)

    dt = lambda n, shp, d=F32, out=False: nc.dram_tensor(
        n, shp, d, kind="ExternalOutput" if out else "ExternalInput").ap()

    # streamed tensors (per-core slices, host-packed)
    egA = dt("egA", [NCHUNK, BL, CHW])      # e_g as [b, l, h], chunked over l
    egB = dt("egB", [NCHUNK, BL, CHW])      # e_g as [b, h, l], chunked over h
    epA = dt("epA", [NCHUNK, BL, CHW])      # e_p as [b, l, h], chunked over l
    gumb = dt("gumb", [n_steps * BL, L])    # gumbel noise per step
    emb = dt("emb", [L * BL, E])            # embedded_inputs flattened [l*b, e]
    # weights / constants
    wi = dt("wi", [E, H4])
    wh = dt("wh", [H, H4])
    wqg = dt("wqg", [H, H])
    wqp = dt("wqp", [H, H])
    bi_r = dt("bi_r", [BL, H4])             # replicated bias rows
    bh_r = dt("bh_r", [BL, H4])
    bqg_r = dt("bqg_r", [BL, H])
    bqp_r = dt("bqp_r", [BL, H])
    vg_r = dt("vg_r", [BL, H])              # v_g replicated rows
    vp_r = dt("vp_r", [BL, H])
    iota_l = dt("iota_l", [BL, L])          # 0..L-1 per row, fp32
    iota_b = dt("iota_b", [BL, 1])          # partition index column, fp32
    ident = dt("ident", [128, 128])         # identity for PE transpose
    xT0 = dt("xT0", [E, BL])                # initial decoder_input^T
    hT0 = dt("hT0", [H, BL])                # initial hx^T
    c0 = dt("c0", [BL, H])                  # initial cx
    # outputs
    probs_o = dt("probs_o", [n_steps * BL, L], out=True)
    sel_o = dt("sel_o", [BL, n_steps], I32, out=True)
    hy_o = dt("hy_o", [BL, H], out=True)
    cy_o = dt("cy_o", [BL, H], out=True)

    with tile.TileContext(nc) as tc, ExitStack() as ctx:
        const = ctx.enter_context(tc.tile_pool(name="const", bufs=1))
        state = ctx.enter_context(tc.tile_pool(name="state", bufs=1))
        stream = ctx.enter_context(tc.tile_pool(name="stream", bufs=stream_bufs))
        work = ctx.enter_context(tc.tile_pool(name="work", bufs=2))
        small = ctx.enter_context(tc.tile_pool(name="small", bufs=2))
        psum = ctx.enter_context(tc.tile_pool(name="psum", bufs=2, space="PSUM"))
        psg = ctx.enter_context(tc.tile_pool(name="psg", bufs=2, space="PSUM"))

        # ---- constants into SBUF ----
        def cload(name, ap_dram, shp, d=F32):
            t = const.tile(shp, d, tag=name)
            nc.sync.dma_start(t[:], ap_dram)
            return t

        WI0 = cload("WI0", wi[0:128, :], [128, H4])
        WI1 = cload("WI1", wi[128:256, :], [128, H4])
        WH0 = cload("WH0", wh[0:128, :], [128, H4])
        WH1 = cload("WH1", wh[128:256, :], [128, H4])
        WQG0 = cload("WQG0", wqg[0:128, :], [128, H])
        WQG1 = cload("WQG1", wqg[128:256, :], [128, H])
        WQP0 = cload("WQP0", wqp[0:128, :], [128, H])
        WQP1 = cload("WQP1", wqp[128:256, :], [128, H])
        BI = cload("BI", bi_r, [BL, H4])
        BH = cload("BH", bh_r, [BL, H4])
        BQG = cload("BQG", bqg_r, [BL, H])
        BQP = cload("BQP", bqp_r, [BL, H])
        VG = cload("VG", vg_r, [BL, H])
        VP = cload("VP", vp_r, [BL, H])
        IOTA = cload("IOTA", iota_l, [BL, L])
        IOTB = cload("IOTB", iota_b, [BL, 1])
        IDENT = cload("IDENT", ident, [128, 128])

        # ---- state tiles (persist across loop iterations) ----
        xTa = state.tile([128, BL], F32, tag="xTa")      # x^T rows e 0..127
        xTb = state.tile([128, BL], F32, tag="xTb")      # x^T rows e 128..255
        hTa = state.tile([128, BL], F32, tag="hTa")
        hTb = state.tile([128, BL], F32, tag="hTb")
        C = state.tile([BL, H], F32, tag="C")
        HY = state.tile([BL, H], F32, tag="HY")
        M = state.tile([BL, L], F32, tag="M")            # additive mask
        PREV = state.tile([BL, 1], F32, tag="PREV")
        SEL = state.tile([BL, n_steps], I32, tag="SEL")
        NEGT = state.tile([BL, L], F32, tag="NEGT")      # -1e30 constant tile

        nc.sync.dma_start(xTa[:], xT0[0:128, :])
        nc.sync.dma_start(xTb[:], xT0[128:256, :])
        nc.sync.dma_start(hTa[:], hT0[0:128, :])
        nc.sync.dma_start(hTb[:], hT0[128:256, :])
        nc.sync.dma_start(C[:], c0)
        nc.vector.memset(M[:], 0.0)
        nc.vector.memset(PREV[:], -1.0)
        nc.vector.memset(NEGT[:], NEG)
        nc.vector.memset(HY[:], 0.0)

        def v3(ap, blk=L_BLK):
            return ap.rearrange("p (l h) -> p l h", l=blk)

        with tc.For_i(0, n_steps, 1) as i:
            # ---------- gumbel noise for this step ----------
            GT = small.tile([BL, L], F32, tag="GT")
            nc.sync.dma_start(GT[:], gumb[bass.ds(i * BL, BL), :])

            # ---------- LSTM: gates = ((x@Wi + bi) + h@Wh) + bh ----------
            G1 = work.tile([BL, H4], F32, tag="G1")
            G3 = work.tile([BL, H4], F32, tag="G3")
            for nb in range(2):
                s5 = bass.ts(nb, 512)
                pg_ = psg.tile([BL, 512], F32, tag="pgates")
                nc.tensor.matmul(pg_[:], xTa[:], WI0[:, s5], start=True, stop=False)
                nc.tensor.matmul(pg_[:], xTb[:], WI1[:, s5], start=False, stop=True)
                nc.vector.tensor_add(G1[:, s5], pg_[:], BI[:, s5])
                ph_ = psg.tile([BL, 512], F32, tag="pgates2")
                nc.tensor.matmul(ph_[:], hTa[:], WH0[:, s5], start=True, stop=False)
                nc.tensor.matmul(ph_[:], hTb[:], WH1[:, s5], start=False, stop=True)
                nc.vector.tensor_add(G1[:, s5], G1[:, s5], ph_[:])
                nc.vector.tensor_add(G3[:, s5], G1[:, s5], BH[:, s5])

            # gate nonlinearities: i,f,o sigmoid (as 0.5*tanh(x/2)+0.5 —
            # measured closest to XLA CPU sigmoid on HW); g tanh
            SIG = work.tile([BL, 3 * H], F32, tag="SIG")   # [i_s | f_s | o_s]
            if use_act_sigmoid:
                for gi, oi in [(0, 0), (1, 1), (3, 2)]:
                    nc.scalar.activation(SIG[:, bass.ts(oi, H)], G3[:, bass.ts(gi, H)], AF.Sigmoid)
            else:
                nc.scalar.activation(SIG[:, 0:512], G3[:, 0:512], AF.Tanh, scale=0.5)
                nc.scalar.activation(SIG[:, 512:768], G3[:, 768:1024], AF.Tanh, scale=0.5)
                nc.vector.tensor_scalar(SIG[:], SIG[:], 0.5, 0.5,
                                        op0=OP.mult, op1=OP.add)
            TG = small.tile([BL, H], F32, tag="TG")
            nc.scalar.activation(TG[:], G3[:, bass.ts(2, H)], AF.Tanh)

            # cy = sig(f)*c + sig(i)*tanh(g); hy = sig(o)*tanh(cy)
            T1 = small.tile([BL, H], F32, tag="T1")
            T2 = small.tile([BL, H], F32, tag="T2")
            nc.vector.tensor_mul(T1[:], SIG[:, bass.ts(1, H)], C[:])
            nc.vector.tensor_mul(T2[:], SIG[:, bass.ts(0, H)], TG[:])
            nc.vector.tensor_add(C[:], T1[:], T2[:])
            TCY = small.tile([BL, H], F32, tag="TCY")
            nc.scalar.activation(TCY[:], C[:], AF.Tanh)
            nc.vector.tensor_mul(HY[:], SIG[:, bass.ts(2, H)], TCY[:])

            # hyT for qg matmul and next-step Wh matmul
            for k, dst in ((0, hTa), (1, hTb)):
                pt = psum.tile([128, BL], F32, tag="ptrans")
                nc.tensor.transpose(pt[:], HY[:, bass.ts(k, 128)], IDENT[:])
                nc.vector.tensor_copy(dst[:], pt[:])

            # qg = hy @ Wq_g + bq_g
            QG = small.tile([BL, H], F32, tag="QG")
            pq = psum.tile([BL, H], F32, tag="pq")
            nc.tensor.matmul(pq[:], hTa[:], WQG0[:], start=True, stop=False)
            nc.tensor.matmul(pq[:], hTb[:], WQG1[:], start=False, stop=True)
            nc.vector.tensor_add(QG[:], pq[:], BQG[:])

            # ---------- mask update: M += (iota == prev) * -1e30 ----------
            DEL = small.tile([BL, L], F32, tag="DEL")
            nc.vector.scalar_tensor_tensor(DEL[:], IOTA[:], PREV[:, 0:1], NEGT[:],
                                           op0=OP.is_equal, op1=OP.mult)
            nc.vector.tensor_add(M[:], M[:], DEL[:])

            # ---------- glimpse: ug = sum_h v_g * tanh(qg + e_g) ----------
            UG = work.tile([BL, L], F32, tag="UG")
            qg_b = QG[:].unsqueeze(1).broadcast_to([BL, L_BLK, H])
            vg_b = VG[:].unsqueeze(1).broadcast_to([BL, L_BLK, H])
            for ci in range(NCHUNK):
                ET = stream.tile([BL, CHW], F32, tag="ET")
                nc.sync.dma_start(ET[:], egA[ci])
                nc.vector.tensor_tensor(v3(ET[:]), v3(ET[:]), qg_b, op=OP.add)
                nc.scalar.activation(ET[:], ET[:], AF.Tanh)
                nc.vector.tensor_tensor(v3(ET[:]), v3(ET[:]), vg_b, op=OP.mult)
                nc.vector.reduce_sum(UG[:, bass.ts(ci, L_BLK)], v3(ET[:]),
                                     axis=mybir.AxisListType.X)

            # ---------- masked softmax over l ----------
            UGM = small.tile([BL, L], F32, tag="UGM")
            nc.vector.tensor_add(UGM[:], UG[:], M[:])
            MX = small.tile([BL, 4], F32, tag="MX")
            nc.vector.reduce_max(MX[:, 0:1], UGM[:], axis=mybir.AxisListType.X)
            nc.vector.tensor_scalar_mul(MX[:, 1:2], MX[:, 0:1], -1.0)
            PEXP = small.tile([BL, L], F32, tag="PEXP")
            SE = small.tile([BL, 4], F32, tag="SE")
            nc.scalar.activation(PEXP[:], UGM[:], AF.Exp, bias=MX[:, 1:2], accum_out=SE[:, 0:1])
            nc.vector.reciprocal(SE[:, 1:2], SE[:, 0:1])
            PG = small.tile([BL, L], F32, tag="PG")
            nc.vector.tensor_scalar_mul(PG[:], PEXP[:], SE[:, 1:2])

            # ---------- gvec = sum_l pg * e_g  (layout [b, h, l]) ----------
            GV = small.tile([BL, H], F32, tag="GV")
            pg_b = PG[:].unsqueeze(1).broadcast_to([BL, L_BLK, L])
            for cj in range(NCHUNK):
                E2 = stream.tile([BL, CHW], F32, tag="ET")
                nc.sync.dma_start(E2[:], egB[cj])
                nc.vector.tensor_tensor(v3(E2[:]), v3(E2[:]), pg_b, op=OP.mult)
                nc.vector.reduce_sum(GV[:, bass.ts(cj, L_BLK)], v3(E2[:]),
                                     axis=mybir.AxisListType.X)

            # ---------- qp = gvec @ Wq_p + bq_p ----------
            gvTa = small.tile([128, BL], F32, tag="gvTa")
            gvTb = small.tile([128, BL], F32, tag="gvTb")
            for k, dst in ((0, gvTa), (1, gvTb)):
                pt2 = psum.tile([128, BL], F32, tag="ptrans")
                nc.tensor.transpose(pt2[:], GV[:, bass.ts(k, 128)], IDENT[:])
                nc.vector.tensor_copy(dst[:], pt2[:])
            QP = small.tile([BL, H], F32, tag="QP")
            pq2 = psum.tile([BL, H], F32, tag="pq")
            nc.tensor.matmul(pq2[:], gvTa[:], WQP0[:], start=True, stop=False)
            nc.tensor.matmul(pq2[:], gvTb[:], WQP1[:], start=False, stop=True)
            nc.vector.tensor_add(QP[:], pq2[:], BQP[:])

            # ---------- pointer: up = sum_h v_p * tanh(qp + e_p) ----------
            UP = work.tile([BL, L], F32, tag="UG")
            qp_b = QP[:].unsqueeze(1).broadcast_to([BL, L_BLK, H])
            vp_b = VP[:].unsqueeze(1).broadcast_to([BL, L_BLK, H])
            for ci in range(NCHUNK):
                EP = stream.tile([BL, CHW], F32, tag="ET")
                nc.sync.dma_start(EP[:], epA[ci])
                nc.vector.tensor_tensor(v3(EP[:]), v3(EP[:]), qp_b, op=OP.add)
                nc.scalar.activation(EP[:], EP[:], AF.Tanh)
                nc.vector.tensor_tensor(v3(EP[:]), v3(EP[:]), vp_b, op=OP.mult)
                nc.vector.reduce_sum(UP[:, bass.ts(ci, L_BLK)], v3(EP[:]),
                                     axis=mybir.AxisListType.X)

            # ---------- logits = C_TANH * tanh(up), mask, log_softmax ----------
            LG = small.tile([BL, L], F32, tag="LG")
            nc.scalar.activation(LG[:], UP[:], AF.Tanh)
            nc.vector.tensor_scalar_mul(LG[:], LG[:], C_TANH)
            nc.vector.tensor_add(LG[:], LG[:], M[:])
            MX2 = small.tile([BL, 4], F32, tag="MX2")
            nc.vector.reduce_max(MX2[:, 0:1], LG[:], axis=mybir.AxisListType.X)
            nc.vector.tensor_scalar_mul(MX2[:, 1:2], MX2[:, 0:1], -1.0)
            SH = small.tile([BL, L], F32, tag="SH")
            nc.vector.tensor_scalar_add(SH[:], LG[:], MX2[:, 1:2])
            ESH = small.tile([BL, L], F32, tag="ESH")
            SE2 = small.tile([BL, 4], F32, tag="SE2")
            nc.scalar.activation(ESH[:], SH[:], AF.Exp, accum_out=SE2[:, 0:1])
            nc.scalar.activation(SE2[:, 1:2], SE2[:, 0:1], AF.Ln)
            nc.vector.tensor_scalar_mul(SE2[:, 2:3], SE2[:, 1:2], -1.0)
            LOGP = small.tile([BL, L], F32, tag="LOGP")
            nc.vector.tensor_scalar_add(LOGP[:], SH[:], SE2[:, 2:3])
            # probs output
            PR = small.tile([BL, L], F32, tag="PR")
            nc.scalar.activation(PR[:], LOGP[:], AF.Exp)
            nc.sync.dma_start(probs_o[bass.ds(i * BL, BL), :], PR[:])

            # ---------- sample: argmax(logp + gumbel), first occurrence ----------
            Y = small.tile([BL, L], F32, tag="Y")
            nc.vector.tensor_add(Y[:], LOGP[:], GT[:])
            MV8 = small.tile([BL, 8], F32, tag="MV8")
            MI8 = small.tile([BL, 8], U32, tag="MI8")
            nc.vector.max(MV8[:], Y[:])
            nc.vector.max_index(MI8[:], MV8[:], Y[:])
            # prev, sel, flat gather index
            nc.vector.tensor_copy(PREV[:, 0:1], MI8[:, 0:1])
            nc.vector.tensor_copy(SEL[:, bass.ds(i, 1)], MI8[:, 0:1])
            FIF = small.tile([BL, 1], F32, tag="FIF")
            nc.vector.scalar_tensor_tensor(FIF[:], PREV[:, 0:1], float(BL), IOTB[:],
                                           op0=OP.mult, op1=OP.add)
            FI = small.tile([BL, 1], I32, tag="FI")
            nc.vector.tensor_copy(FI[:], FIF[:])
            # gather x_next = embedded[idx, b, :]
            XN = small.tile([BL, E], F32, tag="XN")
            nc.gpsimd.indirect_dma_start(
                out=XN[:], out_offset=None, in_=emb,
                in_offset=bass.IndirectOffsetOnAxis(ap=FI[:, 0:1], axis=0))
            for k, dst in ((0, xTa), (1, xTb)):
                pt3 = psum.tile([128, BL], F32, tag="ptrans")
                nc.tensor.transpose(pt3[:], XN[:, bass.ts(k, 128)], IDENT[:])
                nc.vector.tensor_copy(dst[:], pt3[:])

        # ---------- epilogue ----------
        nc.sync.dma_start(hy_o, HY[:])
        nc.sync.dma_start(cy_o, C[:])
        nc.sync.dma_start(sel_o, SEL[:])

    nc.compile()
    return nc


# ---------------------------------------------------------------------------
# host-side packing
# ---------------------------------------------------------------------------

def pack_core(core, e_g, e_p, gumb, inputs, n_steps=L):
    s = slice(core * BL, (core + 1) * BL)
    eg = e_g[:, s, :]                       # [L, BL, H]
    ep = e_p[:, s, :]
    egA = np.ascontiguousarray(
        eg.transpose(1, 0, 2).reshape(BL, NCHUNK, CHW).transpose(1, 0, 2))
    epA = np.ascontiguousarray(
        ep.transpose(1, 0, 2).reshape(BL, NCHUNK, CHW).transpose(1, 0, 2))
    egB = np.ascontiguousarray(
        eg.transpose(1, 2, 0).reshape(BL, NCHUNK, CHW).transpose(1, 0, 2))
    gmb = np.ascontiguousarray(gumb[:n_steps, s, :].reshape(n_steps * BL, L))
    emb = np.ascontiguousarray(
        inputs["embedded_inputs"][:, s, :].reshape(L * BL, E))
    m = {
        "egA": egA, "egB": egB, "epA": epA, "gumb": gmb, "emb": emb,
        "wi": inputs["Wi"], "wh": inputs["Wh"],
        "wqg": inputs["Wq_g"], "wqp": inputs["Wq_p"],
        "bi_r": np.tile(inputs["bi"], (BL, 1)),
        "bh_r": np.tile(inputs["bh"], (BL, 1)),
        "bqg_r": np.tile(inputs["bq_g"], (BL, 1)),
        "bqp_r": np.tile(inputs["bq_p"], (BL, 1)),
        "vg_r": np.tile(inputs["v_g"], (BL, 1)),
        "vp_r": np.tile(inputs["v_p"], (BL, 1)),
        "iota_l": np.tile(np.arange(L, dtype=np.float32), (BL, 1)),
        "iota_b": np.arange(BL, dtype=np.float32).reshape(BL, 1),
        "ident": np.eye(128, dtype=np.float32),
        "xT0": np.ascontiguousarray(inputs["decoder_input"][s].T),
        "hT0": np.ascontiguousarray(inputs["hx"][s].T),
        "c0": np.ascontiguousarray(inputs["cx"][s]),
    }
    return {k: np.ascontiguousarray(v.astype(v.dtype, copy=False)) for k, v in m.items()}


def host_precompute(inputs):
    """e_g / e_p / gumbel exactly as the reference computes them (jax CPU)."""
    import jax
    import jax.numpy as jnp
    cpu = jax.devices("cpu")[0]
    with jax.default_device(cpu):
        ctx = jnp.asarray(inputs["context"])
        e_g = np.asarray(jnp.einsum("lbh,hk->lbk", ctx, jnp.asarray(inputs["Wr_g"]))
                         + jnp.asarray(inputs["br_g"]))
        e_p = np.asarray(jnp.einsum("lbh,hk->lbk", ctx, jnp.asarray(inputs["Wr_p"]))
                         + jnp.asarray(inputs["br_p"]))
        keys = jax.random.split(jax.random.key(42), L)
        gumb = np.stack([np.asarray(jax.random.gumbel(k, (B, L), jnp.float32))
                         for k in keys])
    return e_g, e_p, gumb


def run_cores(nc, in_maps, **kw):
    return run_bass_kernel_spmd(nc, in_maps, list(range(len(in_maps))), **kw)


def kernel(**inputs):
    inputs = {k: np.asarray(v) for k, v in inputs.items()}
    e_g, e_p, gumb = host_precompute(inputs)
    in_maps = [pack_core(c, e_g, e_p, gumb, inputs) for c in range(NCORES)]
    nc = build_program()
    res = run_cores(nc, in_maps)
    probs = np.concatenate(
        [r["probs_o"].reshape(L, BL, L) for r in res.results], axis=1)
    sel = np.concatenate([r["sel_o"].T for r in res.results], axis=1)
    hy = np.concatenate([r["hy_o"] for r in res.results], axis=0)
    cy = np.concatenate([r["cy_o"] for r in res.results], axis=0)
    return probs, sel.astype(np.int32), hy, cy
